# revision 4
# baseline (speedup 1.0000x reference)
"""Trainium2 Bass kernel for nn_AxisNetwork (embedding_lookup + sine MLP).

Math per point (x, y):
    e = lerp(emb0, x) * lerp(emb1, y)          # [256]
    h = sin(30*(e @ w0.T + b0))                # [128]
    h = sin(30*(h @ w1.T + b1))                # [128]
    out = h @ w2.T + b2                        # [3]

Device strategy (pure data parallel over 8 cores, B = N/8 points each):
  * Per-point linear interpolation is replaced by a lookup into a U=32x
    upsampled table (host-precomputed from emb0/emb1, fp16). Snapping to
    the nearest 1/32 sub-cell gives ~5e-4 rel error.
  * dma_gather WITHOUT transpose fetches one contiguous 512-B fp16 row
    per point per axis (row j of a chunk lands whole in partition j%128,
    free slot j//128).  This keeps every DMA descriptor at 512 B - the
    transpose-gather variant degenerates to 2-byte partition writes and
    runs ~90x slower on HW.
  * DVE forms e = e0*e1 in the gathered [point, d] layout; the tensor
    engine then transposes each [128 pt, 128 d] block via identity
    matmuls (fp16, PSUM out) and DVE/ACT copy the transposed blocks back
    to SBUF for the MLP matmuls, where points stream as columns.
  * ACT applies sin(30*z+30*b) via its scale/bias fold; layer-1 args are
    wrapped into [-pi, pi] on DVE first (ACT's Sin spline range).
  * Points are processed in a "wrapped" order j (idx list order), chosen
    so every DMA in the index pipeline moves contiguous >=2 KB rows; the
    host undoes the permutation at the end.
"""

import os

import numpy as np

N_FULL = 1 << 20
NCORES = 8
B = int(os.environ.get("KERNEL_B", N_FULL // NCORES))  # points per core
RES = 512
ED = 256
HID = 128
NOUT = 3
W0_FREQ = 30.0

UPS = 32                  # upsample factor for the snap tables
NROWS = (RES - 1) * UPS   # 16352 valid rows
NROWS_PAD = 16384

CHUNK = 4096              # points per gather chunk
STAGE = 512               # points per compute stage
N_CHUNKS = B // CHUNK
STAGES_PER_CHUNK = CHUNK // STAGE

P = 128
SUB = STAGE // P          # 128-point subtiles per stage (4)
FPP = B // P              # points per partition in the natural layout (1024)

_cache = {}


def _build_nc():
    import concourse.bacc as bacc
    import concourse.bass as bass
    import concourse.mybir as mybir
    import concourse.tile as tile
    from concourse import library_config

    f32 = mybir.dt.float32
    f16 = mybir.dt.float16
    i16 = mybir.dt.int16
    Alu = mybir.AluOpType
    Act = mybir.ActivationFunctionType

    nc = bacc.Bacc("TRN2", target_bir_lowering=False, debug=False,
                   num_devices=NCORES)

    coords_d = nc.dram_tensor("coords", [B, 2], f32, kind="ExternalInput")
    up0_d = nc.dram_tensor("up0", [NROWS_PAD, ED], f16, kind="ExternalInput")
    up1_d = nc.dram_tensor("up1", [NROWS_PAD, ED], f16, kind="ExternalInput")
    w0t_d = nc.dram_tensor("w0t", [2, P, HID], f16, kind="ExternalInput")
    w1t_d = nc.dram_tensor("w1t", [HID, HID], f16, kind="ExternalInput")
    w2t_d = nc.dram_tensor("w2t", [HID, NOUT], f16, kind="ExternalInput")
    b0s_d = nc.dram_tensor("b0s", [P, 1], f32, kind="ExternalInput")
    b1s_d = nc.dram_tensor("b1s", [P, 1], f32, kind="ExternalInput")
    b2t_d = nc.dram_tensor("b2t", [P, SUB * NOUT], f32, kind="ExternalInput")
    ident_d = nc.dram_tensor("ident", [P, P], f16, kind="ExternalInput")
    out_d = nc.dram_tensor("out", [P, (B // P) * NOUT], f32,
                           kind="ExternalOutput")
    # scratch for the index wrap: [axis, point] in natural order
    xybuf = nc.dram_tensor("xybuf", [2, B], i16)

    AFF = 255.5 * UPS         # (0.5c+0.5)*511*UPS == c*AFF + AFF

    with tile.TileContext(nc) as tc:
        with (
            tc.tile_pool(name="const", bufs=1) as cpool,
            tc.tile_pool(name="prep", bufs=1) as prep,
            tc.tile_pool(name="idx", bufs=1) as idxp,
            tc.tile_pool(name="gath", bufs=2) as gath,
            tc.tile_pool(name="eTp", bufs=2) as eTp,
            tc.tile_pool(name="act", bufs=2) as actp,
            tc.tile_pool(name="psT", bufs=2, space="PSUM") as psT_pool,
            tc.tile_pool(name="psA", bufs=2, space="PSUM") as psA,
            tc.tile_pool(name="psB", bufs=2, space="PSUM") as psB,
        ):
            nc.gpsimd.load_library(library_config.mlp)

            # ---- constants / weights ----
            w0t = cpool.tile([P, 2, HID], f16)       # [k, c, m]
            nc.sync.dma_start(out=w0t[:], in_=w0t_d[:].rearrange("c k m -> k c m"))
            w1t = cpool.tile([HID, HID], f16)
            nc.sync.dma_start(out=w1t[:], in_=w1t_d[:])
            w2t = cpool.tile([HID, NOUT], f16)
            nc.sync.dma_start(out=w2t[:], in_=w2t_d[:])
            b0s = cpool.tile([P, 1], f32)
            nc.sync.dma_start(out=b0s[:], in_=b0s_d[:])
            b1s = cpool.tile([P, 1], f32)
            nc.sync.dma_start(out=b1s[:], in_=b1s_d[:])
            b2t = cpool.tile([P, SUB * NOUT], f32)
            nc.sync.dma_start(out=b2t[:], in_=b2t_d[:])
            ident = cpool.tile([P, P], f16)
            nc.sync.dma_start(out=ident[:], in_=ident_d[:])

            out_acc = cpool.tile([P, (B // P) * NOUT], f32)

            # ---- index prep ----
            # natural layout: partition p holds points p*FPP + i, i in
            # [0, FPP); every DMA below moves 2 KB+ contiguous rows.
            ctile = prep.tile([P, FPP, 2], f32)
            nc.sync.dma_start(
                out=ctile[:], in_=coords_d[:].rearrange("(p i) a -> p i a", p=P))
            cflat = ctile[:].rearrange("p i a -> p (i a)")
            cl = prep.tile([P, FPP * 2], f32)
            nc.vector.tensor_scalar(out=cl[:], in0=cflat, scalar1=0.999,
                                    scalar2=-1.0, op0=Alu.min, op1=Alu.max)
            av = prep.tile([P, FPP * 2], f32)
            nc.vector.tensor_scalar(out=av[:], in0=cl[:], scalar1=AFF,
                                    scalar2=AFF, op0=Alu.mult, op1=Alu.add)
            # f32 -> i16 round + (i a) -> (a i) reorder in one convert
            idx16 = prep.tile([P, 2, FPP], i16)
            nc.vector.tensor_copy(
                out=idx16[:].rearrange("p a i -> p i a"),
                in_=av[:].rearrange("p (i a) -> p i a", a=2))
            # contiguous spill: xybuf[a][p*FPP + i]
            for a in range(2):
                nc.sync.dma_start(
                    out=xybuf[a].rearrange("(p i) -> p i", p=P),
                    in_=idx16[:, a, :])
            # reload wrapped-by-16 (gather position j = f*16 + q reads
            # idx[q, f]; here t[q, g*FPP + i] = idx(point (q+16g)*FPP+i)),
            # replicated into all 8 partition groups for the 8 Q7 cores.
            idxs = []
            for a in range(2):
                t = idxp.tile([P, B // 16], i16, tag=f"idxs{a}")
                for g in range(8):
                    nc.sync.dma_start(
                        out=t[16 * g:16 * (g + 1), :].rearrange(
                            "q (g i) -> q g i", i=FPP),
                        in_=xybuf[a].rearrange("(g q i) -> q g i",
                                               q=16, i=FPP))
                idxs.append(t)

            # ---- main pipeline ----
            for k in range(N_CHUNKS):
                g0 = gath.tile([P, CHUNK // P, ED], f16, tag="g0")
                g1 = gath.tile([P, CHUNK // P, ED], f16, tag="g1")
                ncol = CHUNK // 16
                nc.gpsimd.dma_gather(
                    g0[:], up0_d[:], idxs[0][:, k * ncol:(k + 1) * ncol],
                    num_idxs=CHUNK, num_idxs_reg=CHUNK, elem_size=ED,
                    transpose=False, single_packet=False)
                nc.gpsimd.dma_gather(
                    g1[:], up1_d[:], idxs[1][:, k * ncol:(k + 1) * ncol],
                    num_idxs=CHUNK, num_idxs_reg=CHUNK, elem_size=ED,
                    transpose=False, single_packet=False)
                ee = gath.tile([P, CHUNK // P, ED], f16, tag="ee")
                nc.vector.tensor_tensor(
                    out=ee[:].rearrange("p t d -> p (t d)"),
                    in0=g0[:].rearrange("p t d -> p (t d)"),
                    in1=g1[:].rearrange("p t d -> p (t d)"),
                    op=Alu.mult)

                for si in range(STAGES_PER_CHUNK):
                    # transpose 4 subtiles x 2 d-halves: [pt, d] -> [d, pt]
                    eT_ps = psT_pool.tile([P, SUB, 2, P], f16, tag="eT_ps")
                    for t in range(SUB):
                        for c in range(2):
                            nc.tensor.transpose(
                                eT_ps[:, t, c, :],
                                ee[:, si * SUB + t, c * P:(c + 1) * P],
                                ident[:])
                    eTs = eTp.tile([P, 2, STAGE], f16, tag="eTs")
                    cp_in = eT_ps[:].rearrange("p t c n -> p c t n")
                    cp_out = eTs[:].rearrange("p c (t n) -> p c t n", n=P)
                    if si % 3 == 2:
                        nc.scalar.activation(out=cp_out, in_=cp_in,
                                             func=Act.Copy)
                    else:
                        nc.vector.tensor_copy(out=cp_out, in_=cp_in)

                    # layer 0: z0[h, n] = sum_d w0[h, d] eT[d, n]
                    z0 = psA.tile([P, STAGE], f32, tag="z0")
                    for c in range(2):
                        nc.tensor.matmul(
                            z0[:], w0t[:, c, :], eTs[:, c, :],
                            start=(c == 0), stop=(c == 1))
                    h0 = actp.tile([P, STAGE], f16, tag="h0")
                    nc.scalar.activation(out=h0[:], in_=z0[:], func=Act.Sin,
                                         bias=b0s[:], scale=W0_FREQ)
                    # layer 1 (w1t pre-scaled by 30 on the host; wrap
                    # 30*z1+30*b1 into ACT Sin's [-pi, pi] spline range)
                    z1 = psB.tile([P, STAGE], f32, tag="zb")
                    nc.tensor.matmul(z1[:], w1t[:], h0[:],
                                     start=True, stop=True)
                    t1 = actp.tile([P, STAGE], f32, tag="t1")
                    nc.vector.add_range_wrap(out=t1[:], in_=z1[:],
                                             shift=b1s[:],
                                             bound=float(np.pi),
                                             period=float(2 * np.pi))
                    h1 = actp.tile([P, STAGE], f16, tag="h1")
                    nc.scalar.activation(out=h1[:], in_=t1[:], func=Act.Sin)
                    # layer 2 (points become the stationary M dim)
                    o2 = psB.tile([P, SUB * NOUT], f32, tag="zb")
                    for t in range(SUB):
                        nc.tensor.matmul(
                            o2[:, t * NOUT:(t + 1) * NOUT],
                            h1[:, t * P:(t + 1) * P],
                            w2t[:],
                            start=True, stop=True)
                    s = k * STAGES_PER_CHUNK + si
                    nc.vector.scalar_tensor_tensor(
                        out=out_acc[:, s * SUB * NOUT:(s + 1) * SUB * NOUT],
                        in0=o2[:], scalar=1.0, in1=b2t[:],
                        op0=Alu.mult, op1=Alu.add)

            nc.sync.dma_start(out=out_d[:], in_=out_acc[:])

    nc.compile()
    return nc


def _host_prep(inputs):
    coords = np.ascontiguousarray(inputs["coords"], dtype=np.float32)
    emb0 = np.asarray(inputs["emb0"], dtype=np.float32)
    emb1 = np.asarray(inputs["emb1"], dtype=np.float32)
    w0 = np.asarray(inputs["w0"], dtype=np.float32)
    b0 = np.asarray(inputs["b0"], dtype=np.float32)
    w1 = np.asarray(inputs["w1"], dtype=np.float32)
    b1 = np.asarray(inputs["b1"], dtype=np.float32)
    w2 = np.asarray(inputs["w2"], dtype=np.float32)
    b2 = np.asarray(inputs["b2"], dtype=np.float32)

    def upsample(emb):
        i = np.arange(RES - 1)
        w = (np.arange(UPS, dtype=np.float64) / UPS).astype(np.float32)
        t = (1.0 - w)[None, :, None] * emb[i][:, None, :] \
            + w[None, :, None] * emb[i + 1][:, None, :]
        t = t.reshape(NROWS, ED)
        pad = np.zeros((NROWS_PAD - NROWS, ED), np.float32)
        return np.concatenate([t, pad], 0).astype(np.float16)

    up0 = upsample(emb0)
    up1 = upsample(emb1)
    w0t = np.ascontiguousarray(
        w0.T.reshape(2, P, HID)).astype(np.float16)        # [c, k, m]
    w1t = np.ascontiguousarray(w1.T * W0_FREQ).astype(np.float16)  # pre-scaled
    w2t = np.ascontiguousarray(w2.T).astype(np.float16)    # [k, 3]
    b0s = (W0_FREQ * b0).reshape(P, 1).astype(np.float32)
    b1s = (W0_FREQ * b1).reshape(P, 1).astype(np.float32)
    b2t = np.tile(b2, SUB).reshape(1, -1).repeat(P, 0).astype(np.float32)
    ident = np.eye(P, dtype=np.float16)

    shared = dict(up0=up0, up1=up1, w0t=w0t, w1t=w1t, w2t=w2t,
                  b0s=b0s, b1s=b1s, b2t=b2t, ident=ident)
    in_maps = []
    for c in range(NCORES):
        shard = np.ascontiguousarray(coords[c * B:(c + 1) * B])
        in_maps.append(dict(coords=shard, **shared))
    return in_maps


def _gather_perm():
    """point id n for each gather position j (see index prep)."""
    j = np.arange(B)
    q = j % 16
    f = j // 16
    g = f // FPP
    i = f % FPP
    return (q + 16 * g) * FPP + i


last_results = None


def kernel(**inputs):
    global last_results
    from concourse.bass_utils import run_bass_kernel_spmd
    import os

    if "nc" not in _cache:
        _cache["nc"] = _build_nc()
    nc = _cache["nc"]

    in_maps = _host_prep(inputs)
    trace = bool(int(os.environ.get("KERNEL_TRACE", "0")))
    res = run_bass_kernel_spmd(nc, in_maps, core_ids=list(range(NCORES)),
                               trace=trace)
    last_results = res

    perm = _gather_perm()
    outs = []
    for c in range(NCORES):
        dev = res.results[c]["out"]                  # [128, (B/128)*3]
        flat_j = dev.reshape(P, B // P, NOUT).transpose(1, 0, 2).reshape(
            B, NOUT)                                 # indexed by position j
        out_n = np.empty_like(flat_j)
        out_n[perm] = flat_j
        outs.append(out_n)
    return np.ascontiguousarray(
        np.concatenate(outs, 0).astype(np.float32))


# revision 11
# speedup vs baseline: 1.4444x; 1.4444x over previous
"""Trainium2 Bass kernel for nn_AxisNetwork (embedding_lookup + sine MLP).

Math per point (x, y):
    e = lerp(emb0, x) * lerp(emb1, y)          # [256]
    h = sin(30*(e @ w0.T + b0))                # [128]
    h = sin(30*(h @ w1.T + b1))                # [128]
    out = h @ w2.T + b2                        # [3]

Device strategy (pure data parallel over 8 cores, B = N/8 points each).

A per-point table gather is descriptor-bound on TRN2: SWDGE generates one
DMA descriptor per gathered row on the GPSIMD Q7 (~8 ns each), which
serializes at ~2 ms/core.  Instead the interpolation itself is done on the
tensor engine with NO gather at all:

  * The host sorts each core's points by x-cell into chunks of 8192, then
    by y-cell within each chunk, and greedily packs consecutive points
    into 512-point STAGES whose x- and y-index windows each fit in 44
    table rows (the input's x/y correlation makes this cheap: ~270 stages
    per core, ~5% padding).
  * Per stage the host slices the ORIGINAL (non-upsampled) 512x256
    embedding tables to the stage's windows -> winx/winy [44, 256] f16,
    and emits per-point window-relative coordinates: AC5 rows
    [i0x', wx, i0y', wy, 1] (f16; i0' is an exact small integer).
  * On device, one k=5 matmul builds M[r, n] = ac'_n - r for all 88
    window rows; DVE computes u = |M|-1, ACT computes tent = relu(-u)
    = relu(1 - |ac' - r|) -- the exact linear-interpolation weights
    (tent basis reproduces piecewise-linear interp exactly).
  * Interp = tent matmuls: e_axis[d, n] = win[., d].T @ tent (k=44).
    DVE multiplies the two axes' PSUM results into ee [256, 512] f16.
  * Sine MLP as before: z0 = w0 @ ee (k=256), ACT sin(30 z + 30 b);
    z1 with w1 pre-scaled by 30, DVE wraps into [-pi, pi] (ACT's Sin
    spline range, verified on HW), sin; z2 with points as the stationary
    dim; DVE adds b2 and the host undoes the sort permutation.
"""

import os

import numpy as np

N_FULL = 1 << 20
NCORES = 8
B = N_FULL // NCORES      # points per core
RES = 512
ED = 256
HID = 128
NOUT = 3
W0_FREQ = 30.0

CHUNK = 8192              # x-sort chunk (window fitting granularity)
STAGE = 512               # points per compute stage
WX = 63                   # x tent rows (chunk x-span <= 35 verified)
WY = 63                   # y tent rows (stages greedily cut to span <= 61)
WT = 128                  # [63 x-tents, x-sum-row, 63 y-tents, y-sum-row]
GRP = 16                  # stages per AC5 load
WGRP = 4                  # stages per window load

P = 128
SUB = STAGE // P

_cache = {}


def _build_nc(s_tot):
    import concourse.bacc as bacc
    import concourse.bass as bass
    import concourse.mybir as mybir
    import concourse.tile as tile

    f32 = mybir.dt.float32
    f16 = mybir.dt.float16
    Alu = mybir.AluOpType
    Act = mybir.ActivationFunctionType

    BS = s_tot * STAGE    # padded point slots per core

    nc = bacc.Bacc("TRN2", target_bir_lowering=False, debug=False,
                   num_devices=NCORES)

    ac5_d = nc.dram_tensor("ac5", [5, BS], f16, kind="ExternalInput")
    win_d = nc.dram_tensor("win", [s_tot, WT, ED], f16, kind="ExternalInput")
    aff_d = nc.dram_tensor("aff", [5, WT], f16, kind="ExternalInput")
    w0t_d = nc.dram_tensor("w0t", [2, P, HID], f16, kind="ExternalInput")
    w1t_d = nc.dram_tensor("w1t", [HID, HID], f16, kind="ExternalInput")
    w2t_d = nc.dram_tensor("w2t", [HID, NOUT], f16, kind="ExternalInput")
    b0s_d = nc.dram_tensor("b0s", [P, 1], f32, kind="ExternalInput")
    b1s_d = nc.dram_tensor("b1s", [P, 1], f32, kind="ExternalInput")
    b2t_d = nc.dram_tensor("b2t", [P, SUB * NOUT], f32, kind="ExternalInput")
    out_d = nc.dram_tensor("out", [P, (BS // P) * NOUT], f32,
                           kind="ExternalOutput")

    with tile.TileContext(nc) as tc:
        with (
            tc.tile_pool(name="const", bufs=1) as cpool,
            tc.tile_pool(name="acp", bufs=2) as acp,
            tc.tile_pool(name="winp", bufs=2) as winp,
            tc.tile_pool(name="tp", bufs=2) as tp,
            tc.tile_pool(name="act", bufs=2) as actp,
            tc.tile_pool(name="psM", bufs=2, space="PSUM") as psM,
            tc.tile_pool(name="psE", bufs=1, space="PSUM") as psE,
            tc.tile_pool(name="psA", bufs=1, space="PSUM") as psA,
            tc.tile_pool(name="psB", bufs=1, space="PSUM") as psB,
        ):
            # ---- constants / weights ----
            aff = cpool.tile([5, WT], f16)
            nc.sync.dma_start(out=aff[:], in_=aff_d[:])
            w0t = cpool.tile([P, 2, HID], f16)       # [k, c, m]
            nc.sync.dma_start(out=w0t[:], in_=w0t_d[:].rearrange("c k m -> k c m"))
            w1t = cpool.tile([HID, HID], f16)
            nc.sync.dma_start(out=w1t[:], in_=w1t_d[:])
            w2t = cpool.tile([HID, NOUT], f16)
            nc.sync.dma_start(out=w2t[:], in_=w2t_d[:])
            b0s = cpool.tile([P, 1], f32)
            nc.sync.dma_start(out=b0s[:], in_=b0s_d[:])
            b1s = cpool.tile([P, 1], f32)
            nc.sync.dma_start(out=b1s[:], in_=b1s_d[:])
            b2t = cpool.tile([P, SUB * NOUT], f32)
            nc.sync.dma_start(out=b2t[:], in_=b2t_d[:])

            out_acc = cpool.tile([P, (BS // P) * NOUT], f32)

            for s in range(s_tot):
                if s % GRP == 0:
                    ac5t = acp.tile([5, GRP * STAGE], f16, tag="ac5")
                    nc.sync.dma_start(
                        out=ac5t[:],
                        in_=ac5_d[:, s * STAGE:(s + GRP) * STAGE])
                if s % WGRP == 0:
                    w4 = winp.tile([WT, WGRP, ED], f16, tag="w4")
                    nc.sync.dma_start(
                        out=w4[:],
                        in_=win_d[s:s + WGRP].rearrange("s r d -> r s d"))
                off = (s % GRP) * STAGE
                wi = s % WGRP

                # tent args: M[r, n] = ac'_n - r for 88 window rows
                M2 = psM.tile([WT, STAGE], f32, tag="m2")
                nc.tensor.matmul(M2[:], aff[:], ac5t[:, off:off + STAGE],
                                 start=True, stop=True)
                # v = min(|M|, 1); tent = 1 - v is folded into the window
                # tables (negated rows + sum row against the const-2 column)
                u = tp.tile([WT, STAGE], f16, tag="u")
                nc.scalar.activation(out=u[:], in_=M2[:], func=Act.Abs)
                v = tp.tile([WT, STAGE], f16, tag="v")
                nc.vector.tensor_scalar(out=v[:], in0=u[:], scalar1=1.0,
                                        scalar2=0.0, op0=Alu.min,
                                        op1=Alu.add)

                # interpolation: e_axis[d, n] = win'[., d].T @ v
                e0 = psE.tile([P, 2, STAGE], f32, tag="e0")
                e1 = psE.tile([P, 2, STAGE], f32, tag="e1")
                for h in range(2):
                    nc.tensor.matmul(e0[:, h, :],
                                     w4[0:64, wi, h * P:(h + 1) * P],
                                     v[0:64, :], start=True, stop=True)
                    nc.tensor.matmul(e1[:, h, :],
                                     w4[64:128, wi, h * P:(h + 1) * P],
                                     v[64:128, :], start=True, stop=True)
                # DVE cannot read two PSUM operands: stage e0 through SBUF
                s0 = tp.tile([P, 2, STAGE], f16, tag="s0")
                ee = tp.tile([P, 2, STAGE], f16, tag="ee")
                for h in range(2):
                    nc.scalar.activation(out=s0[:, h, :], in_=e0[:, h, :],
                                         func=Act.Copy)
                    nc.vector.tensor_tensor(
                        out=ee[:, h, :], in0=s0[:, h, :], in1=e1[:, h, :],
                        op=Alu.mult)

                # layer 0: z0[h, n] = sum_d w0[h, d] ee[d, n]
                z0 = psA.tile([P, STAGE], f32, tag="z0")
                for c in range(2):
                    nc.tensor.matmul(z0[:], w0t[:, c, :], ee[:, c, :],
                                     start=(c == 0), stop=(c == 1))
                h0 = actp.tile([P, STAGE], f16, tag="h0")
                nc.scalar.activation(out=h0[:], in_=z0[:], func=Act.Sin,
                                     bias=b0s[:], scale=W0_FREQ)
                # layer 1 (w1t pre-scaled by 30; wrap into ACT Sin's range)
                z1 = psB.tile([P, STAGE], f32, tag="zb")
                nc.tensor.matmul(z1[:], w1t[:], h0[:], start=True, stop=True)
                t1 = actp.tile([P, STAGE], f32, tag="t1")
                nc.vector.add_range_wrap(out=t1[:], in_=z1[:], shift=b1s[:],
                                         bound=float(np.pi),
                                         period=float(2 * np.pi))
                h1 = actp.tile([P, STAGE], f16, tag="h1")
                nc.scalar.activation(out=h1[:], in_=t1[:], func=Act.Sin)
                # layer 2 (points become the stationary M dim)
                o2 = psB.tile([P, SUB * NOUT], f32, tag="zb")
                for t in range(SUB):
                    nc.tensor.matmul(
                        o2[:, t * NOUT:(t + 1) * NOUT],
                        h1[:, t * P:(t + 1) * P],
                        w2t[:], start=True, stop=True)
                nc.vector.scalar_tensor_tensor(
                    out=out_acc[:, s * SUB * NOUT:(s + 1) * SUB * NOUT],
                    in0=o2[:], scalar=1.0, in1=b2t[:],
                    op0=Alu.mult, op1=Alu.add)

            nc.sync.dma_start(out=out_d[:], in_=out_acc[:])

    nc.compile()
    return nc


def _plan_core(pts):
    """Sort/bucket one core's points; returns the stage plan."""
    acx = (0.5 * np.clip(pts[:, 0].astype(np.float64), -1.0, 0.999) + 0.5) \
        * (RES - 1)
    acy = (0.5 * np.clip(pts[:, 1].astype(np.float64), -1.0, 0.999) + 0.5) \
        * (RES - 1)
    i0x = np.floor(acx).astype(np.int64)
    i0y = np.floor(acy).astype(np.int64)
    wx = acx - i0x
    wy = acy - i0y

    stages = []   # (point_idx_array, basex, basey)
    order1 = np.argsort(i0x, kind="stable")
    for k in range(B // CHUNK):
        seg = order1[k * CHUNK:(k + 1) * CHUNK]
        bx = int(i0x[seg].min())
        assert int(i0x[seg].max()) - bx + 1 <= WX, "x window overflow"
        seg2 = seg[np.argsort(i0y[seg], kind="stable")]
        sy = i0y[seg2]
        i = 0
        n = len(sy)
        while i < n:
            j = min(i + STAGE, n)
            while sy[j - 1] - sy[i] + 1 > WY - 2:
                j = i + np.searchsorted(sy[i:j], sy[i] + WY - 2,
                                        side="right")
            stages.append((seg2[i:j], bx, int(sy[i])))
            i = j
    return stages, i0x, i0y, wx, wy


def _host_prep(inputs):
    coords = np.ascontiguousarray(inputs["coords"], dtype=np.float32)
    emb0 = np.asarray(inputs["emb0"], dtype=np.float32)
    emb1 = np.asarray(inputs["emb1"], dtype=np.float32)
    w0 = np.asarray(inputs["w0"], dtype=np.float32)
    b0 = np.asarray(inputs["b0"], dtype=np.float32)
    w1 = np.asarray(inputs["w1"], dtype=np.float32)
    b1 = np.asarray(inputs["b1"], dtype=np.float32)
    w2 = np.asarray(inputs["w2"], dtype=np.float32)
    b2 = np.asarray(inputs["b2"], dtype=np.float32)

    plans = []
    max_stages = 0
    for c in range(NCORES):
        plan = _plan_core(coords[c * B:(c + 1) * B])
        plans.append(plan)
        max_stages = max(max_stages, len(plan[0]))
    lcm = np.lcm(GRP, WGRP)
    s_tot = int(-(-max_stages // lcm) * lcm)

    emb0h = emb0.astype(np.float16)
    emb1h = emb1.astype(np.float16)
    w0t = np.ascontiguousarray(
        w0.T.reshape(2, P, HID)).astype(np.float16)        # [c, k, m]
    w1t = np.ascontiguousarray(w1.T * W0_FREQ).astype(np.float16)
    w2t = np.ascontiguousarray(w2.T).astype(np.float16)
    b0s = (W0_FREQ * b0).reshape(P, 1).astype(np.float32)
    b1s = (W0_FREQ * b1).reshape(P, 1).astype(np.float32)
    b2t = np.tile(b2, SUB).reshape(1, -1).repeat(P, 0).astype(np.float32)
    aff = np.zeros((5, WT), np.float32)
    aff[0, :WX] = 1.0
    aff[1, :WX] = 1.0
    aff[4, :WX] = -np.arange(WX)
    aff[4, 63] = 2.0
    aff[2, 64:64 + WY] = 1.0
    aff[3, 64:64 + WY] = 1.0
    aff[4, 64:64 + WY] = -np.arange(WY)
    aff[4, 127] = 2.0
    aff = aff.astype(np.float16)

    shared = dict(aff=aff, w0t=w0t, w1t=w1t, w2t=w2t,
                  b0s=b0s, b1s=b1s, b2t=b2t)
    in_maps = []
    perms = []
    BS = s_tot * STAGE
    for c in range(NCORES):
        stages, i0x, i0y, wx, wy = plans[c]
        ac5 = np.zeros((5, BS), np.float32)
        ac5[4] = 1.0
        win = np.zeros((s_tot, WT, ED), np.float16)
        pos = np.full(BS, -1, np.int64)     # position -> original point
        for s, (idx, bx, by) in enumerate(stages):
            n = len(idx)
            sl = slice(s * STAGE, s * STAGE + n)
            ac5[0, sl] = i0x[idx] - bx
            ac5[1, sl] = wx[idx]
            ac5[2, sl] = i0y[idx] - by
            ac5[3, sl] = wy[idx]
            pos[s * STAGE:s * STAGE + n] = idx
            nrx = min(WX, RES - bx)
            win[s, :nrx] = -emb0h[bx:bx + nrx]
            win[s, 63] = emb0.astype(np.float64)[bx:bx + nrx].sum(0).astype(
                np.float16)
            nry = min(WY, RES - by)
            win[s, 64:64 + nry] = -emb1h[by:by + nry]
            win[s, 127] = emb1.astype(np.float64)[by:by + nry].sum(0).astype(
                np.float16)
        in_maps.append(dict(ac5=ac5.astype(np.float16), win=win, **shared))
        perms.append(pos)
    return in_maps, perms, s_tot


last_results = None


def kernel(**inputs):
    global last_results
    from concourse.bass_utils import run_bass_kernel_spmd
    import os

    in_maps, perms, s_tot = _host_prep(inputs)
    key = ("nc", s_tot)
    if key not in _cache:
        _cache[key] = _build_nc(s_tot)
    nc = _cache[key]

    trace = bool(int(os.environ.get("KERNEL_TRACE", "0")))
    res = run_bass_kernel_spmd(nc, in_maps, core_ids=list(range(NCORES)),
                               trace=trace)
    last_results = res

    BS = s_tot * STAGE
    outs = []
    for c in range(NCORES):
        dev = res.results[c]["out"]                  # [128, (BS/128)*3]
        flat = dev.reshape(P, BS // P, NOUT).transpose(1, 0, 2).reshape(
            BS, NOUT)                                # indexed by position
        pos = perms[c]
        valid = pos >= 0
        out_c = np.empty((B, NOUT), flat.dtype)
        out_c[pos[valid]] = flat[valid]
        outs.append(out_c)
    return np.ascontiguousarray(
        np.concatenate(outs, 0).astype(np.float32))
